# revision 1
# baseline (speedup 1.0000x reference)
"""Trainium2 Bass kernel for BilinearInteraction.

Computes out[b,p,:] = (x[:,pi[p],:] @ W[p]) * x[:,pj[p],:] for all P=276
field pairs (pi,pj) = combinations(24, 2), B=2048, E=128.

Strategy (8 NeuronCores):
  - Data-parallel: shard batch dim (2048 -> 256 rows per core), replicate W.
  - All inputs pre-arranged on host into SBUF-layout 2D arrays and cast to
    bf16 (free on host; halves DMA bytes).  W (9 MB bf16) stays resident in
    SBUF for the whole kernel.
  - Pairs sharing the same first field i are contiguous in p, so one
    stationary operand (x_i^T, [E=128 x 128 batch]) serves a whole group of
    matmuls whose moving operand is a contiguous slice of W.
  - The post-matmul elementwise work (downcast + multiply by x_j) is split
    across three engines per pair-group:
      * DIRECT: VectorE tensor_tensor straight from fp32 PSUM (1x mode) —
        fuses downcast+multiply in one pass, no ScalarE involvement;
      * ACT+DVE: ScalarE copies PSUM->SBUF with bf16 downcast, VectorE
        multiplies in bf16 (2x mode);
      * ACT+GPS: same copy, GpSimd does the bf16 multiply (SBUF only).
  - Input DMAs ride the Activation HWDGE ring, output DMAs the SP ring, so
    ~2.3 MB output stores never queue behind the 12 MB of input loads.

The kernel is HBM-bandwidth bound (~30 MB/core => ~75 us of DMA); PE, ACT,
DVE and GPSIMD are balanced to ~45-55 us each underneath it.
"""

import numpy as np
import ml_dtypes

# ---------------------------------------------------------------- constants
F = 24          # fields
E = 128         # embedding dim
B = 2048        # batch
P = F * (F - 1) // 2        # 276 pairs
NCORES = 8
B_LOCAL = B // NCORES       # 256 rows per core
BCH = B_LOCAL // 128        # 2 batch chunks of 128
COLS = P * E                # 35328 output columns per batch chunk

# group g = pairs whose first field is g; sizes 23, 22, ..., 1
NGROUPS = F - 1
GS = [F - 1 - g for g in range(NGROUPS)]                  # pairs per group
GP = [0]
for s in GS:
    GP.append(GP[-1] + s)                                 # pair start per group

# stages: contiguous runs of whole groups, ~2.2 MB of bf16 output DMA per
# (batch chunk, stage) — few large stores measured the best DMA rate.
STAGE_G = [(0, 3), (3, 7), (7, 12), (12, NGROUPS)]

PSUM_TILE = 2048            # fp32 elems per partition = 4 banks
BANK = 512                  # fp32 elems per PSUM bank
MM_MAX = 512                # max matmul free dim (one fp32 PSUM bank)

# elementwise path per group: fraction of columns handled by each path
#   DIRECT  : DVE tensor_tensor from PSUM (no ACT copy).  Safe only with
#             drain-priority emission: DVE's PSUM drains must be queued
#             ahead of the previous stage's bf16 TTs or the PE stalls.
#   COPY_DVE: ACT copy -> DVE bf16 TT
#   COPY_GPS: ACT copy -> GPSIMD bf16 TT (measured too slow and serial)
DIRECT, COPY_DVE, COPY_GPS = 0, 1, 2
PATH_W = {DIRECT: 0.25, COPY_DVE: 0.75, COPY_GPS: 0.0}


def _assign_paths():
    """Deterministic weighted assignment of a path to each group."""
    paths = {}
    acc = {k: 0.0 for k in PATH_W}
    done = {k: 0.0 for k in PATH_W}
    total = 0.0
    for g in range(NGROUPS):
        w = GS[g]
        # pick path with largest deficit (target*total_so_far - done)
        total += w
        best, bestd = None, None
        for k in PATH_W:
            d = PATH_W[k] * total - done[k]
            if bestd is None or d > bestd:
                best, bestd = k, d
        paths[g] = best
        done[best] += w
    return paths


GROUP_PATH = _assign_paths()


def _build_schedule():
    """Static per-batch-chunk schedule (see v1 docstring for fields)."""
    stages = []
    for (glo, ghi) in STAGE_G:
        pair0 = GP[glo]
        npairs = GP[ghi] - GP[glo]
        col0 = pair0 * E
        cols = npairs * E

        gb = [(GP[g] - pair0) * E for g in range(glo, ghi)] + [cols]
        groups = [(g, (GP[g] - pair0) * E, GS[g] * E) for g in range(glo, ghi)]

        ptiles = []
        pt0 = 0
        while pt0 < cols:
            pcols = min(PSUM_TILE, cols - pt0)
            cuts = set()
            c = pt0
            while c < pt0 + pcols:
                cuts.add(c)
                c += BANK
            for b in gb:
                if pt0 < b < pt0 + pcols:
                    cuts.add(b)
            cuts = sorted(cuts) + [pt0 + pcols]
            segs = []
            for k in range(len(cuts) - 1):
                lcol0, n = cuts[k], cuts[k + 1] - cuts[k]
                g = None
                for gi, gl0, gc in groups:
                    if gl0 <= lcol0 < gl0 + gc:
                        g = gi
                        break
                assert g is not None and n <= MM_MAX
                span = (lcol0 - pt0) // BANK
                assert (lcol0 - pt0 + n - 1) // BANK == span
                segs.append([lcol0, n, g, span])
            out_segs = []
            for k, (lcol0, n, g, span) in enumerate(segs):
                first = k == 0 or segs[k - 1][3] != span
                last = k == len(segs) - 1 or segs[k + 1][3] != span
                out_segs.append((lcol0, n, g, first, last))

            # chunks: (group x this tile) intersections, with path tags
            chunks = []
            for gi, gl0, gc in groups:
                lo = max(gl0, pt0)
                hi = min(gl0 + gc, pt0 + pcols)
                if lo < hi:
                    chunks.append((gi, lo, hi, GROUP_PATH[gi]))
            # maximal runs of copied (non-DIRECT) chunks for merged ACT copies
            runs = []
            cur = None
            for (gi, lo, hi, path) in chunks:
                if path == DIRECT:
                    if cur is not None:
                        runs.append(cur)
                        cur = None
                else:
                    if cur is None:
                        cur = [lo, hi]
                    else:
                        cur[1] = hi
            if cur is not None:
                runs.append(cur)

            ptiles.append(dict(pt0=pt0, pcols=pcols, segs=out_segs,
                               chunks=chunks, copy_runs=runs))
            pt0 += pcols

        stages.append(
            dict(pair0=pair0, npairs=npairs, col0=col0, cols=cols,
                 ptiles=ptiles, groups=groups)
        )
    return stages


STAGES = _build_schedule()

_NC = None


def _build_module():
    """Build (once) the Bass module: same program for all 8 cores."""
    global _NC
    if _NC is not None:
        return _NC

    import concourse.bass as bass
    import concourse.tile as tile
    from concourse import bacc, mybir

    bf = mybir.dt.bfloat16
    f32 = mybir.dt.float32

    nc = bacc.Bacc("TRN2", target_bir_lowering=False, debug=False)

    xT = nc.declare_dram_parameter("xT", [128, F * B_LOCAL], bf, isOutput=False)
    xn = nc.declare_dram_parameter("xn", [128, BCH * F * E], bf, isOutput=False)
    Wt = nc.declare_dram_parameter("Wt", [128, COLS], bf, isOutput=False)
    out = nc.declare_dram_parameter("out", [B_LOCAL, COLS], bf, isOutput=True)

    with tile.TileContext(nc) as tc:
        with (
            tc.tile_pool(name="const", bufs=1) as cpool,
            tc.tile_pool(name="mm", bufs=2) as mmpool,
            tc.tile_pool(name="so", bufs=2) as sopool,
            tc.tile_pool(name="ps", bufs=2, space=bass.MemorySpace.PSUM) as pspool,
        ):
            # ---- all DMA on the single SP HWDGE ring (two active rings
            # measured ~10% slower aggregate).  Order: xT, then just enough
            # W for the first psum tile, so matmuls start ~14us in.
            # xT split: stage 0 only needs groups 0-2's stationary columns
            # (first 3*256 = 768), so the first matmul starts ~5us earlier.
            xT_sb = cpool.tile([128, F * B_LOCAL], bf, tag="xT")
            nc.sync.dma_start(out=xT_sb[:, 0:768], in_=xT[:, 0:768])

            w_sb = []
            for s, st in enumerate(STAGES):
                w = cpool.tile([128, st["cols"]], bf, tag=f"w{s}", name=f"w{s}")
                w_sb.append(w)

            def load_w(s, lo, hi):
                nc.sync.dma_start(
                    out=w_sb[s][:, lo:hi],
                    in_=Wt[:, STAGES[s]["col0"] + lo: STAGES[s]["col0"] + hi],
                )

            # head: xT + first psum tile of stage-0 W => first matmul ~14us;
            # remaining W loads are interleaved with the output stores below
            # so stores don't queue behind 9 MB of W on the FIFO ring.
            h0 = STAGES[0]["ptiles"][0]["pcols"]
            load_w(0, 0, h0)
            nc.sync.dma_start(
                out=xT_sb[:, 768: F * B_LOCAL], in_=xT[:, 768: F * B_LOCAL]
            )
            load_w(0, h0, STAGES[0]["cols"])
            xn_sb = cpool.tile([128, BCH * F * E], bf, tag="xn")
            nc.sync.dma_start(out=xn_sb[:], in_=xn[:])
            load_w(1, 0, STAGES[1]["cols"])

            # bf16 TTs + the store of each stage-instance are deferred until
            # after the NEXT instance's PSUM work is enqueued, so DVE's FIFO
            # always services PSUM drains first and the PE never waits on
            # bf16 multiply work.
            pending = []

            def flush_pending():
                while pending:
                    pending.pop(0)()

            for bc in range(BCH):
                for s, st in enumerate(STAGES):
                    mm_t = mmpool.tile([128, st["cols"]], bf, tag="mm")
                    so_t = sopool.tile([128, st["cols"]], bf, tag="so")
                    for pti, pt in enumerate(st["ptiles"]):
                        ps = pspool.tile([128, pt["pcols"]], f32, tag="ps")
                        for (lcol0, n, g, first, last) in pt["segs"]:
                            lhsT = xT_sb[
                                :, g * B_LOCAL + bc * 128:
                                g * B_LOCAL + bc * 128 + 128
                            ]
                            rhs = w_sb[s][:, lcol0: lcol0 + n]
                            nc.tensor.matmul(
                                ps[:, lcol0 - pt["pt0"]: lcol0 - pt["pt0"] + n],
                                lhsT, rhs, start=first, stop=last,
                            )
                        # merged ACT copies for the copied groups in this tile
                        for (lo, hi) in pt["copy_runs"]:
                            nc.scalar.copy(
                                out=mm_t[:, lo:hi],
                                in_=ps[:, lo - pt["pt0"]: hi - pt["pt0"]],
                            )
                        # direct path: DVE TT straight from PSUM per chunk
                        for (gi, lo, hi, path) in pt["chunks"]:
                            if path != DIRECT:
                                continue
                            gl0 = (GP[gi] - st["pair0"]) * E
                            xoff = bc * F * E + (gi + 1) * E + (lo - gl0)
                            nc.vector.tensor_mul(
                                so_t[:, lo:hi],
                                ps[:, lo - pt["pt0"]: hi - pt["pt0"]],
                                xn_sb[:, xoff: xoff + (hi - lo)],
                            )
                    flush_pending()

                    def deferred(bc=bc, s=s, st=st, mm_t=mm_t, so_t=so_t):
                        for (g, glcol0, gcols) in st["groups"]:
                            if GROUP_PATH[g] == DIRECT:
                                continue
                            nc.vector.tensor_mul(
                                so_t[:, glcol0: glcol0 + gcols],
                                mm_t[:, glcol0: glcol0 + gcols],
                                xn_sb[:, bc * F * E + (g + 1) * E:
                                      bc * F * E + (g + 1) * E + gcols],
                            )
                        nc.sync.dma_start(
                            out=out[bc * 128: (bc + 1) * 128,
                                    st["col0"]: st["col0"] + st["cols"]],
                            in_=so_t[:],
                        )
                        # interleave the remaining W loads between the first
                        # stores so neither blocks the other on the FIFO ring
                        if bc == 0 and s + 2 < len(STAGES):
                            load_w(s + 2, 0, STAGES[s + 2]["cols"])

                    pending.append(deferred)
            flush_pending()

    nc.compile()
    _NC = nc
    return nc


def _prep_inputs(x, W):
    """Host-side shard + relayout + bf16 cast. Returns in_maps for 8 cores."""
    bf = ml_dtypes.bfloat16
    x = np.ascontiguousarray(x, dtype=np.float32)
    W = np.ascontiguousarray(W, dtype=np.float32)

    # Wt[e, p*128+f] = W[p, e, f]
    Wt = np.ascontiguousarray(W.transpose(1, 0, 2).reshape(128, COLS)).astype(bf)

    in_maps = []
    for c in range(NCORES):
        xs = x[c * B_LOCAL: (c + 1) * B_LOCAL]            # [256, 24, 128]
        # xT[e, f*256+b] = xs[b, f, e]
        xT = np.ascontiguousarray(
            xs.transpose(2, 1, 0).reshape(128, F * B_LOCAL)
        ).astype(bf)
        # xn[b, bc*3072 + f*128 + e] = xs[bc*128+b, f, e]
        xn = np.ascontiguousarray(
            xs.reshape(BCH, 128, F, E).transpose(1, 0, 2, 3).reshape(128, BCH * F * E)
        ).astype(bf)
        in_maps.append({"xT": xT, "xn": xn, "Wt": Wt})
    return in_maps


def run_on_hw(x, W, trace=False, **run_kwargs):
    """Run the kernel on the 8 NeuronCores; returns (output fp32, results)."""
    from concourse.bass_utils import run_bass_kernel_spmd

    nc = _build_module()
    in_maps = _prep_inputs(x, W)
    res = run_bass_kernel_spmd(
        nc, in_maps, list(range(NCORES)), trace=trace, **run_kwargs
    )
    shards = []
    for c in range(NCORES):
        o = np.asarray(res.results[c]["out"])
        shards.append(o.astype(np.float32).reshape(B_LOCAL, P, E))
    return np.concatenate(shards, axis=0), res


def kernel(x, W):
    import os
    try:
        out, _ = run_on_hw(x, W, trace=False)
    except Exception:
        # transient device wedge (e.g. NRT_EXEC_UNIT_UNRECOVERABLE):
        # retry once with a core reset
        os.environ["NEURON_RT_RESET_CORES"] = "1"
        out, _ = run_on_hw(x, W, trace=False)
    return out

